# revision 2
# baseline (speedup 1.0000x reference)
"""Instance-norm kernel for TRN2 (Bass/Tile), 8-core data-parallel.

Problem: ten (64, 3, 512, 512) f32; per-(n,c) mean and unbiased std over
(H, W); out = (x - mean) / (sqrt(var_unbiased) + 1e-8).

Sharding: batch dim N=64 split across 8 cores -> 8 batches (24 images)
per core. Each 512x512 image is viewed as an SBUF tile [128, 2048]:
  - per-partition mean/var via bn_stats/bn_aggr (DVE)
  - cross-partition reduce + broadcast in one PE matmul with a ones
    [128, 128] stationary: psum[p, :] = column sums of [m_p, E_p[x^2]]
  - scalar chain -> per-partition mean and 1/(std+eps)
  - apply (x - mean) * rstd in one DVE tensor_scalar pass
  - loads on the SP HWDGE ring, stores on the ACT HWDGE ring so the two
    streams' fixed costs overlap.
"""

from contextlib import ExitStack

import numpy as np

import concourse.bass as bass
import concourse.tile as tile
from concourse import bacc, mybir
from concourse._compat import with_exitstack
from concourse.bass_utils import run_bass_kernel_spmd

N, C, H, W = 64, 3, 512, 512
NCORES = 8
NB = N // NCORES              # batches per core
IMGS = NB * C                 # images (n,c) per core
HW = H * W                    # 262144 elements per image
P = 128                       # SBUF partitions
F = HW // P                   # 2048 free elements per partition
EPS = 1e-8
BN_FMAX = 512
NSUB = F // BN_FMAX           # bn_stats subgroups per partition

FP32 = mybir.dt.float32


@with_exitstack
def _norm_body(ctx: ExitStack, tc: tile.TileContext, y: bass.AP, x: bass.AP):
    nc = tc.nc
    data = ctx.enter_context(tc.tile_pool(name="data", bufs=4))
    small = ctx.enter_context(tc.tile_pool(name="small", bufs=4))
    psum = ctx.enter_context(tc.tile_pool(name="psum", bufs=4, space="PSUM"))
    singles = ctx.enter_context(tc.tile_pool(name="singles", bufs=1))

    ones = singles.tile([P, P], FP32)
    nc.vector.memset(ones, 1.0)

    # sqrt(var_b * corr) turns the biased (/HW) variance into the
    # unbiased (/(HW-1)) one.
    corr = float(HW) / float(HW - 1)

    for i in range(IMGS):
        xt = data.tile([P, F], FP32)
        nc.sync.dma_start(out=xt[:], in_=x[i * P : (i + 1) * P, :])

        stats = small.tile([P, NSUB, nc.vector.BN_STATS_DIM], FP32)
        for s in range(NSUB):
            nc.vector.bn_stats(
                out=stats[:, s, :], in_=xt[:, s * BN_FMAX : (s + 1) * BN_FMAX]
            )
        mv = small.tile([P, 2], FP32)
        nc.vector.bn_aggr(out=mv[:], in_=stats[:])

        # mv -> [m_p, E_p[x^2]] so the ones-matmul yields both raw sums.
        msq = small.tile([P, 1], FP32)
        nc.scalar.square(msq[:], mv[:, 0:1])
        nc.vector.tensor_add(mv[:, 1:2], mv[:, 1:2], msq[:])

        ps = psum.tile([P, 2], FP32)
        nc.tensor.matmul(ps[:], ones[:], mv[:], start=True, stop=True)
        # ps[:, 0] = sum_p m_p = P * mean, ps[:, 1] = P * E[x^2], on every
        # partition.

        mean = small.tile([P, 1], FP32)
        nc.scalar.mul(mean[:], ps[:, 0:1], 1.0 / P)
        mean2 = small.tile([P, 1], FP32)
        nc.scalar.activation(
            mean2[:], ps[:, 0:1], func=mybir.ActivationFunctionType.Square,
            scale=1.0 / P,
        )
        varb = small.tile([P, 1], FP32)
        nc.vector.tensor_scalar(
            out=varb[:], in0=ps[:, 1:2], scalar1=1.0 / P, scalar2=mean2[:],
            op0=mybir.AluOpType.mult, op1=mybir.AluOpType.subtract,
        )
        std = small.tile([P, 1], FP32)
        nc.scalar.activation(
            std[:], varb[:], func=mybir.ActivationFunctionType.Sqrt, scale=corr
        )
        stdp = small.tile([P, 1], FP32)
        nc.vector.tensor_scalar_add(stdp[:], std[:], EPS)
        rstd = small.tile([P, 1], FP32)
        nc.vector.reciprocal(rstd[:], stdp[:])

        nc.vector.tensor_scalar(
            out=xt[:], in0=xt[:], scalar1=mean[:], scalar2=rstd[:],
            op0=mybir.AluOpType.subtract, op1=mybir.AluOpType.mult,
        )
        nc.scalar.dma_start(out=y[i * P : (i + 1) * P, :], in_=xt[:])


def _build():
    nc = bacc.Bacc(
        "TRN2", target_bir_lowering=False, debug=False, num_devices=NCORES
    )
    x = nc.dram_tensor("x", [IMGS * P, F], FP32, kind="ExternalInput").ap()
    y = nc.dram_tensor("y", [IMGS * P, F], FP32, kind="ExternalOutput").ap()
    with tile.TileContext(nc) as tc:
        _norm_body(tc, y, x)
    nc.finalize()
    return nc


_nc = None


def _run(ten: np.ndarray, **kw):
    global _nc
    if _nc is None:
        _nc = _build()
    shards = np.ascontiguousarray(ten, dtype=np.float32).reshape(
        NCORES, IMGS * P, F
    )
    in_maps = [{"x": shards[k]} for k in range(NCORES)]
    res = run_bass_kernel_spmd(_nc, in_maps, core_ids=list(range(NCORES)), **kw)
    out = np.stack([res.results[k]["y"] for k in range(NCORES)])
    return out.reshape(N, C, H, W), res


def kernel(**inputs: np.ndarray) -> np.ndarray:
    out, _ = _run(np.asarray(inputs["ten"]))
    return out
